# revision 1
# baseline (speedup 1.0000x reference)
"""KV-cache attention Bass kernel for Trainium2, 8 NeuronCores.

Sharding: batch (4) x query-half (2) -> 8 cores. Each core projects Q for its
1024 query rows, projects the full new K/V for its batch (duplicated across the
core pair), and runs softmax(Q K^T / 8) V over the 8192-row concatenated cache.

Layout strategy (everything kept in matmul-native layouts, no on-device
transposes):
  - scores are computed TRANSPOSED: S^T[t, s] with lhsT = K^T tile, rhs = Q^T.
  - softmax over t (partition dim) uses exp with a constant shift (exact:
    softmax is shift-invariant); the denominator comes from a [2,512] ones
    matmul per score tile, PSUM-accumulated across ALL kv chunks in a single
    bank (two accumulation groups at partition offsets 0 and 32).
  - P^T is exactly the stationary operand layout the PV matmul needs, so no
    transposes are ever required.
All matmul operands are bf16 (Fast Weight Load hides the per-matmul weight
load; fp32 PSUM accumulation). Input DMAs are spread across the sync/scalar/
vector/gpsimd queues so the cold start is not single-queue limited.
"""
import sys
import numpy as np

if "/opt/trn_rl_repo" not in sys.path:
    sys.path.insert(0, "/opt/trn_rl_repo")

import ml_dtypes
import concourse.bacc as bacc
import concourse.mybir as mybir
from concourse.tile import TileContext
from concourse.bass_utils import run_bass_kernel_spmd

B, S_NEW, S_CACHE, D = 4, 2048, 6144, 1024
S_KV = S_CACHE + S_NEW            # 8192
SQ = S_NEW // 2                   # 1024 query rows per core
N_CORES = 8
P = 128
ET = D // P                       # 8 feature tiles
DT = D // P                       # 8 contraction tiles
CHUNK = 512                       # kv rows per chunk
N_CHUNKS = S_KV // CHUNK          # 16 (12 cached + 4 new)
N_CACHED_CHUNKS = S_CACHE // CHUNK
TT4 = CHUNK // P                  # 4 t-ptiles per chunk
SCALE = 0.125                     # 1/sqrt(64)
SHIFT = -16.0                     # constant softmax shift (exact)

F32 = mybir.dt.float32
BF16 = mybir.dt.bfloat16
NPBF16 = np.dtype(ml_dtypes.bfloat16)

# chunk processing order: new-KV chunks (AllGather-dependent) sit at
# positions 6..9 so neither the head nor the tail of phase B waits on them.
CHUNK_SRC = (
    [("c", i) for i in range(6)]
    + [("n", r, l) for r in range(2) for l in range(2)]
    + [("c", i) for i in range(6, N_CACHED_CHUNKS)]
)

_cache = {}


def _build():
    nc = bacc.Bacc("TRN2", target_bir_lowering=False, debug=False,
                   num_devices=N_CORES)
    ht = nc.dram_tensor("ht", [P, DT * SQ], BF16, kind="ExternalInput")
    wq = nc.dram_tensor("wq", [P, DT * D], BF16, kind="ExternalInput")
    wk = nc.dram_tensor("wk", [P, DT * D], BF16, kind="ExternalInput")
    wv = nc.dram_tensor("wv", [P, DT * D], BF16, kind="ExternalInput")
    # chunk-major cache layouts: one chunk's K^T/V is fully contiguous per
    # partition (8KB lines) so each phase-B load runs at full DMA efficiency.
    kcT = nc.dram_tensor("kcT", [N_CACHED_CHUNKS, P, ET, CHUNK], BF16,
                         kind="ExternalInput")
    vc = nc.dram_tensor("vc", [N_CACHED_CHUNKS, P, TT4, D], BF16,
                        kind="ExternalInput")
    bq = nc.dram_tensor("bq", [P, ET], F32, kind="ExternalInput")
    bk = nc.dram_tensor("bk", [P, ET], F32, kind="ExternalInput")
    bv = nc.dram_tensor("bv", [P, D], F32, kind="ExternalInput")
    ident = nc.dram_tensor("ident", [P, P], F32, kind="ExternalInput")
    out = nc.dram_tensor("out", [SQ, D], BF16, kind="ExternalOutput")

    with TileContext(nc) as tc:
        with tc.tile_pool(name="big", bufs=1) as big, \
             tc.tile_pool(name="bias", bufs=1) as biasp, \
             tc.tile_pool(name="spsum", bufs=3, space="PSUM") as spsum, \
             tc.tile_pool(name="dnpsum", bufs=1, space="PSUM") as dnpsum, \
             tc.tile_pool(name="opsum", bufs=2, space="PSUM") as opsum, \
             tc.tile_pool(name="early", bufs=1) as earlyp, \
             tc.tile_pool(name="abig", bufs=1) as abig, \
             tc.tile_pool(name="stage", bufs=4) as stagep, \
             tc.tile_pool(name="kpool", bufs=2) as kpool, \
             tc.tile_pool(name="vpool", bufs=2) as vpool, \
             tc.tile_pool(name="ptpool", bufs=2) as ptpool, \
             tc.tile_pool(name="fin", bufs=4) as finp, \
             tc.tile_pool(name="obig", bufs=1) as obig, \
             tc.tile_pool(name="dram", bufs=1, space="DRAM") as dpool:

            # new-KV scratch, local-chunk-major so gathered loads are
            # contiguous per partition as well.
            nkT_h = dpool.tile([P, 2, ET, CHUNK], BF16, name="nkT_h")
            nv_h = dpool.tile([P, 2, TT4, D], BF16, name="nv_h")
            nkT_g = dpool.tile([2, P, 2, ET, CHUNK], BF16, name="nkT_g")
            nv_g = dpool.tile([2, P, 2, TT4, D], BF16, name="nv_g")

            qT_sb = big.tile([P, ET * SQ], BF16, name="qT_sb")
            kt0_sb = earlyp.tile([P, ET, CHUNK], BF16, name="kt0_sb")
            v0_sb = earlyp.tile([P, TT4, D], BF16, name="v0_sb")
            out_acc = obig.tile([P, SQ // P, D], F32, name="out_acc")
            dn_sb = obig.tile([2, SQ], F32, name="dn_sb")

            # per-dt tiles: fine-grained DMA -> compute dependencies.
            # Only sync/scalar/gpsimd can issue DMA; round-robin wq+ht
            # across all three so phase A1's full contraction (4MB) lands
            # as early as possible, dt-major so group 0 completes first.
            wq_t = [abig.tile([P, D], BF16, name=f"wq{dt}") for dt in range(DT)]
            ht_t = [abig.tile([P, SQ], BF16, name=f"ht{dt}") for dt in range(DT)]
            wk_t = [abig.tile([P, D], BF16, name=f"wk{dt}") for dt in range(DT)]
            wv_t = [abig.tile([P, D], BF16, name=f"wv{dt}") for dt in range(DT)]

            bq_sb = biasp.tile([P, ET], F32, name="bq_sb")
            bk_sb = biasp.tile([P, ET], F32, name="bk_sb")
            bv_sb = biasp.tile([P, D], F32, name="bv_sb")
            sh_sb = biasp.tile([P, 1], F32, name="sh_sb")
            nc.vector.memset(sh_sb[:], SHIFT)
            ones_sb = biasp.tile([P, 2], F32, name="ones_sb")
            nc.vector.memset(ones_sb[:], 1.0)
            onesb_sb = biasp.tile([P, 2], BF16, name="onesb_sb")
            nc.vector.tensor_copy(onesb_sb[:], ones_sb[:])
            id_sb = biasp.tile([P, P], F32, name="id_sb")

            nc.gpsimd.dma_start(out=bq_sb[:], in_=bq[:])
            QS = [nc.sync, nc.scalar, nc.gpsimd]
            for dt in range(DT):  # wq[dt] and ht[dt] on different queues
                QS[dt % 3].dma_start(out=wq_t[dt][:],
                                     in_=wq[:, dt * D:(dt + 1) * D])
                QS[(dt + 1) % 3].dma_start(out=ht_t[dt][:],
                                           in_=ht[:, dt * SQ:(dt + 1) * SQ])
            for dt in range(DT):  # wk needed ~30us in, wv ~55us: all queues
                QS[(dt + 2) % 3].dma_start(out=wk_t[dt][:],
                                           in_=wk[:, dt * D:(dt + 1) * D])
            for dt in range(DT):
                QS[dt % 3].dma_start(out=wv_t[dt][:],
                                     in_=wv[:, dt * D:(dt + 1) * D])
            nc.gpsimd.dma_start(out=id_sb[:], in_=ident[:])
            nc.gpsimd.dma_start(out=bk_sb[:], in_=bk[:])
            nc.gpsimd.dma_start(out=bv_sb[:], in_=bv[:])
            nc.gpsimd.dma_start(out=kt0_sb[:], in_=kcT[0])
            nc.gpsimd.dma_start(out=v0_sb[:], in_=vc[0])

            # ---- Phase A1: Q^T projection, contraction split in two
            # halves so the PE starts dense work before all of wq/ht lands.
            qa_sb = abig.tile([P, ET * SQ], BF16, name="qa_sb")
            SPLIT = 5  # 5+3: part A's PE pace matches DMA arrival; part B
            # starts right as the last wq/ht tiles land (~28us in).
            for et in range(ET):
                for sc in range(SQ // 512):
                    ps = spsum.tile([P, 512], F32, name="ps_q", tag="sp")
                    for dt in range(SPLIT):
                        nc.tensor.matmul(
                            ps[:],
                            wq_t[dt][:, et * P:(et + 1) * P],
                            ht_t[dt][:, sc * 512:(sc + 1) * 512],
                            start=(dt == 0), stop=(dt == SPLIT - 1))
                    nc.scalar.activation(
                        qa_sb[:, et * SQ + sc * 512:et * SQ + (sc + 1) * 512],
                        ps[:], mybir.ActivationFunctionType.Identity,
                        bias=bq_sb[:, et:et + 1])
            for et in range(ET):
                for sc in range(SQ // 512):
                    ps = spsum.tile([P, 512], F32, name="ps_q2", tag="sp")
                    for dt in range(SPLIT, DT):
                        nc.tensor.matmul(
                            ps[:],
                            wq_t[dt][:, et * P:(et + 1) * P],
                            ht_t[dt][:, sc * 512:(sc + 1) * 512],
                            start=(dt == SPLIT), stop=(dt == DT - 1))
                    nc.vector.tensor_add(
                        qT_sb[:, et * SQ + sc * 512:et * SQ + (sc + 1) * 512],
                        qa_sb[:, et * SQ + sc * 512:et * SQ + (sc + 1) * 512],
                        ps[:])

            # ---- Phase A2: new K^T -> DRAM scratch ----
            for et in range(ET):
                for sc in range(SQ // 512):
                    ps = spsum.tile([P, 512], F32, name="ps_k", tag="sp")
                    for dt in range(DT):
                        nc.tensor.matmul(
                            ps[:],
                            wk_t[dt][:, et * P:(et + 1) * P],
                            ht_t[dt][:, sc * 512:(sc + 1) * 512],
                            start=(dt == 0), stop=(dt == DT - 1))
                    st = stagep.tile([P, 512], BF16, name="st_k", tag="stage")
                    nc.scalar.activation(
                        st[:], ps[:], mybir.ActivationFunctionType.Identity,
                        bias=bk_sb[:, et:et + 1])
                    nc.scalar.dma_start(out=nkT_h[:, sc, et, :], in_=st[:])

            nc.gpsimd.collective_compute(
                "AllGather",
                mybir.AluOpType.bypass,
                replica_groups=[[0, 1], [2, 3], [4, 5], [6, 7]],
                ins=[nkT_h[:]],
                outs=[nkT_g[:]])

            # ---- Phase A3: new V -> DRAM scratch ----
            for tt in range(SQ // P):
                for ec in range(D // 512):
                    ps = spsum.tile([P, 512], F32, name="ps_v", tag="sp")
                    for dt in range(DT):
                        nc.tensor.matmul(
                            ps[:],
                            ht_t[dt][:, tt * P:(tt + 1) * P],
                            wv_t[dt][:, ec * 512:(ec + 1) * 512],
                            start=(dt == 0), stop=(dt == DT - 1))
                    st = stagep.tile([P, 512], BF16, name="st_v", tag="stage")
                    nc.vector.tensor_add(st[:], ps[:], bv_sb[:, ec * 512:(ec + 1) * 512])
                    nc.scalar.dma_start(
                        out=nv_h[:, tt // TT4, tt % TT4, ec * 512:(ec + 1) * 512],
                        in_=st[:])

            # ---- pair AllGather of the new K/V halves (overlaps attention
            # on the cached chunks) ----
            nc.gpsimd.collective_compute(
                "AllGather",
                mybir.AluOpType.bypass,
                replica_groups=[[0, 1], [2, 3], [4, 5], [6, 7]],
                ins=[nv_h[:]],
                outs=[nv_g[:]])

            # denominator PSUM: one bank; sb=0 rows at partitions 0:2,
            # sb=1 rows at partitions 32:34 (col-group granularity).
            dn_ps = dnpsum.tile([34, 512], F32, name="dn_ps")

            # ---- Phase B: attention over 16 kv chunks ----
            for c in range(N_CHUNKS):
                src = CHUNK_SRC[c]
                if c == 0:
                    kt_sb, v_sb = kt0_sb, v0_sb
                else:
                    kt_sb = kpool.tile([P, ET, CHUNK], BF16, name="kt_sb")
                    v_sb = vpool.tile([P, TT4, D], BF16, name="v_sb")
                if c == 0:
                    pass
                elif src[0] == "c":
                    ci = src[1]
                    nc.sync.dma_start(out=kt_sb[:], in_=kcT[ci])
                    nc.scalar.dma_start(out=v_sb[:], in_=vc[ci])
                else:
                    rank, lc = src[1], src[2]
                    nc.gpsimd.dma_start(out=kt_sb[:], in_=nkT_g[rank, :, lc])
                    nc.gpsimd.dma_start(out=v_sb[:], in_=nv_g[rank, :, lc])

                for sb in range(SQ // 512):
                    dnp = 0 if sb == 0 else 32
                    pt = ptpool.tile([P, TT4, 512], BF16, name="pt")
                    for tt4 in range(TT4):
                        stp = spsum.tile([P, 512], F32, name="stp", tag="sp")
                        for et in range(ET):
                            nc.tensor.matmul(
                                stp[:],
                                kt_sb[:, et, tt4 * P:(tt4 + 1) * P],
                                qT_sb[:, et * SQ + sb * 512:et * SQ + (sb + 1) * 512],
                                start=(et == 0), stop=(et == ET - 1))
                        nc.scalar.activation(
                            pt[:, tt4, :], stp[:],
                            mybir.ActivationFunctionType.Exp,
                            bias=sh_sb[:], scale=SCALE)
                    for si in range(4):
                        si_g = sb * 4 + si
                        if si == 1:
                            # denominator batch rides between PV groups 0
                            # and 1: every exp is long finished by now.
                            for tt4 in range(TT4):
                                nc.tensor.matmul(
                                    dn_ps[dnp:dnp + 2, :], onesb_sb[:],
                                    pt[:, tt4, :],
                                    start=(c == 0 and tt4 == 0),
                                    stop=(c == N_CHUNKS - 1 and tt4 == TT4 - 1))
                        po = opsum.tile([P, D], F32, name="po")
                        for tt4 in range(TT4):
                            lhs = pt[:, tt4, si * P:(si + 1) * P]
                            st0 = (tt4 == 0)
                            sp1 = (tt4 == TT4 - 1)
                            nc.tensor.matmul(po[:, 0:512], lhs,
                                             v_sb[:, tt4, 0:512],
                                             start=st0, stop=sp1)
                            nc.tensor.matmul(po[:, 512:1024], lhs,
                                             v_sb[:, tt4, 512:1024],
                                             start=st0, stop=sp1)
                        if c == 0:
                            nc.vector.tensor_copy(out_acc[:, si_g, :], po[:])
                        else:
                            nc.vector.tensor_add(out_acc[:, si_g, :],
                                                 out_acc[:, si_g, :], po[:])
                    if c == N_CHUNKS - 1:
                        # this sb's denominators are final: normalize and
                        # stream out its 4 query ptiles while the other sb
                        # (or nothing) still computes. Alternate the scale
                        # between ACT and DVE; rotate stores across queues.
                        nc.vector.tensor_copy(
                            dn_sb[0:2, sb * 512:(sb + 1) * 512],
                            dn_ps[dnp:dnp + 2, :])
                        for si in range(4):
                            si_g = sb * 4 + si
                            tps = spsum.tile([P, 512], F32, name="tps", tag="sp")
                            nc.tensor.matmul(
                                tps[:, 0:2], dn_sb[0:2, si_g * P:(si_g + 1) * P],
                                id_sb[0:2, 0:2], start=True, stop=True)
                            rec = finp.tile([P, 1], F32, name="rec")
                            nc.vector.reciprocal(rec[:], tps[:, 0:1])
                            ost = finp.tile([P, D], BF16, name="ost")
                            if si % 2 == 0:
                                nc.scalar.activation(
                                    ost[:], out_acc[:, si_g, :D],
                                    mybir.ActivationFunctionType.Copy,
                                    scale=rec[:])
                            else:
                                nc.vector.tensor_scalar_mul(
                                    ost[:], out_acc[:, si_g, :D], rec[:])
                            for hh in range(2):
                                QS[(sb * 8 + si * 2 + hh) % 3].dma_start(
                                    out=out[si_g * P:(si_g + 1) * P,
                                            hh * 512:(hh + 1) * 512],
                                    in_=ost[:, hh * 512:(hh + 1) * 512])

    nc.compile()
    return nc


def _prep(hidden_states, cached_key, cached_value, Wq, bq, Wk, bk, Wv, bv):
    """Host-side resharding into SBUF-image layouts (pure reshapes/copies)."""
    def ptile_cols(a):  # [R, C] with R = n*128 -> [128, n*C] (partition-major)
        n = a.shape[0] // P
        return np.ascontiguousarray(
            a.reshape(n, P, a.shape[1]).transpose(1, 0, 2)).reshape(P, -1)

    w_h = {}
    for nm, W in (("wq", Wq), ("wk", Wk), ("wv", Wv)):
        w_h[nm] = ptile_cols(np.ascontiguousarray(W.T)).astype(NPBF16)
    bq_h = np.ascontiguousarray(bq.reshape(ET, P).T)             # [128, 8]
    bk_h = np.ascontiguousarray(bk.reshape(ET, P).T)
    bv_h = np.ascontiguousarray(np.broadcast_to(bv, (P, D)))     # [128, 1024]
    id_h = np.eye(P, dtype=np.float32)

    in_maps = []
    for b in range(B):
        ht_full = ptile_cols(np.ascontiguousarray(hidden_states[b].T))  # [128, 8*2048]
        # chunk-major: [chunk, P, ET, CHUNK] / [chunk, P, TT4, D]
        kcT_h = ptile_cols(np.ascontiguousarray(cached_key[b].T)) \
            .astype(NPBF16).reshape(P, ET, N_CACHED_CHUNKS, CHUNK)
        kcT_h = np.ascontiguousarray(kcT_h.transpose(2, 0, 1, 3))
        vc_h = np.ascontiguousarray(
            cached_value[b].reshape(N_CACHED_CHUNKS, TT4, P, D)
            .transpose(0, 2, 1, 3)).astype(NPBF16)
        for h in range(2):
            ht_v = ht_full.reshape(P, DT, S_NEW)
            ht_c = np.ascontiguousarray(
                ht_v[:, :, h * SQ:(h + 1) * SQ]).reshape(P, DT * SQ) \
                .astype(NPBF16)
            in_maps.append({
                "ht": ht_c, "kcT": kcT_h, "vc": vc_h,
                "wq": w_h["wq"], "wk": w_h["wk"], "wv": w_h["wv"],
                "bq": bq_h, "bk": bk_h, "bv": bv_h, "ident": id_h,
            })
    return in_maps


def kernel(hidden_states, cached_key, cached_value, Wq, bq, Wk, bk, Wv, bv,
           _trace=False):
    if "nc" not in _cache:
        _cache["nc"] = _build()
    nc = _cache["nc"]
    in_maps = _prep(
        np.asarray(hidden_states, dtype=np.float32),
        np.asarray(cached_key, dtype=np.float32),
        np.asarray(cached_value, dtype=np.float32),
        np.asarray(Wq, dtype=np.float32), np.asarray(bq, dtype=np.float32),
        np.asarray(Wk, dtype=np.float32), np.asarray(bk, dtype=np.float32),
        np.asarray(Wv, dtype=np.float32), np.asarray(bv, dtype=np.float32))
    res = run_bass_kernel_spmd(nc, in_maps, list(range(N_CORES)), trace=_trace)
    _cache["last_result"] = res
    out = np.empty((B, S_NEW, D), np.float32)
    for b in range(B):
        for h in range(2):
            out[b, h * SQ:(h + 1) * SQ, :] = \
                res.results[2 * b + h]["out"].astype(np.float32)
    return out



# revision 6
# speedup vs baseline: 1.0647x; 1.0647x over previous
"""KV-cache attention Bass kernel for Trainium2, 8 NeuronCores.

Sharding: batch (4) x query-half (2) -> 8 cores. Each core projects Q for its
1024 query rows, projects the full new K/V for its batch (duplicated across the
core pair), and runs softmax(Q K^T / 8) V over the 8192-row concatenated cache.

Layout strategy (everything kept in matmul-native layouts, no on-device
transposes):
  - scores are computed TRANSPOSED: S^T[t, s] with lhsT = K^T tile, rhs = Q^T.
  - softmax over t (partition dim) uses exp with a constant shift (exact:
    softmax is shift-invariant); the denominator is accumulated on the Pool
    engine (pt_acc += exp tile, fp32) and reduced across partitions at the
    very end with 8 tiny N=1 matmuls -> per-partition reciprocal scale.
  - P^T is exactly the stationary operand layout the PV matmul needs, so no
    transposes are ever required.
All matmul operands are bf16 (Fast Weight Load hides the per-matmul weight
load; fp32 PSUM accumulation).

Phase schedule (PE never idles on DMA/collectives):
  A1 (Q^T) | B0 B1 | A2 (new K^T -> DRAM, pair AllGather) | B2 B3 |
  A3 (new V -> DRAM, pair AllGather) | B4..B11 | new-KV chunks 12..15 | tail.
Cold-start DMA uses only the two HWDGE rings (sync/scalar, ~0.6us fixed cost)
rather than the SWDGE gpsimd ring (~2us fixed); gpsimd is reserved for
biases, collectives and the Pool-engine denominator accumulation.
"""
import sys
import numpy as np

if "/opt/trn_rl_repo" not in sys.path:
    sys.path.insert(0, "/opt/trn_rl_repo")

import ml_dtypes
import concourse.bacc as bacc
import concourse.mybir as mybir
from concourse.tile import TileContext
from concourse.bass_utils import run_bass_kernel_spmd

B, S_NEW, S_CACHE, D = 4, 2048, 6144, 1024
S_KV = S_CACHE + S_NEW            # 8192
SQ = S_NEW // 2                   # 1024 query rows per core
N_CORES = 8
P = 128
ET = D // P                       # 8 feature tiles
DT = D // P                       # 8 contraction tiles
CHUNK = 512                       # kv rows per chunk
N_CHUNKS = S_KV // CHUNK          # 16 (12 cached + 4 new)
N_CACHED_CHUNKS = S_CACHE // CHUNK
TT4 = CHUNK // P                  # 4 t-ptiles per chunk
SCALE = 0.125                     # 1/sqrt(64)
SHIFT = -16.0                     # constant softmax shift (exact)

F32 = mybir.dt.float32
BF16 = mybir.dt.bfloat16
NPBF16 = np.dtype(ml_dtypes.bfloat16)

# chunk processing order: cached chunks first, AllGather-dependent new-KV
# chunks at the very end (the gathers complete ~200us before they are needed).
CHUNK_SRC = (
    [("c", i) for i in range(N_CACHED_CHUNKS)]
    + [("n", r, l) for r in range(2) for l in range(2)]
)

_cache = {}


def _build():
    nc = bacc.Bacc("TRN2", target_bir_lowering=False, debug=False,
                   num_devices=N_CORES)
    ht = nc.dram_tensor("ht", [P, DT * SQ], BF16, kind="ExternalInput")
    wq = nc.dram_tensor("wq", [P, DT * D], BF16, kind="ExternalInput")
    wk = nc.dram_tensor("wk", [P, DT * D], BF16, kind="ExternalInput")
    wv = nc.dram_tensor("wv", [P, DT * D], BF16, kind="ExternalInput")
    # chunk-major cache layouts: one chunk's K^T/V is fully contiguous per
    # partition (8KB lines) so each phase-B load runs at full DMA efficiency.
    kcT = nc.dram_tensor("kcT", [N_CACHED_CHUNKS, P, ET, CHUNK], BF16,
                         kind="ExternalInput")
    vc = nc.dram_tensor("vc", [N_CACHED_CHUNKS, P, TT4, D], BF16,
                        kind="ExternalInput")
    bq = nc.dram_tensor("bq", [P, ET], F32, kind="ExternalInput")
    bk = nc.dram_tensor("bk", [P, ET], F32, kind="ExternalInput")
    bv = nc.dram_tensor("bv", [P, D], F32, kind="ExternalInput")
    out = nc.dram_tensor("out", [SQ, D], BF16, kind="ExternalOutput")

    with TileContext(nc) as tc:
        with tc.tile_pool(name="big", bufs=1) as big, \
             tc.tile_pool(name="bias", bufs=1) as biasp, \
             tc.tile_pool(name="spsum", bufs=3, space="PSUM") as spsum, \
             tc.tile_pool(name="dnpsum", bufs=1, space="PSUM") as dnpsum, \
             tc.tile_pool(name="opsum", bufs=2, space="PSUM") as opsum, \
             tc.tile_pool(name="early", bufs=1) as earlyp, \
             tc.tile_pool(name="abig", bufs=1) as abig, \
             tc.tile_pool(name="stage", bufs=4) as stagep, \
             tc.tile_pool(name="kpool", bufs=2) as kpool, \
             tc.tile_pool(name="vpool", bufs=2) as vpool, \
             tc.tile_pool(name="ptpool", bufs=2) as ptpool, \
             tc.tile_pool(name="fin", bufs=4) as finp, \
             tc.tile_pool(name="obig", bufs=1) as obig, \
             tc.tile_pool(name="dram", bufs=1, space="DRAM") as dpool:

            # new-KV scratch, local-chunk-major so gathered loads are
            # contiguous per partition as well.
            nkT_h = dpool.tile([P, 2, ET, CHUNK], BF16, name="nkT_h")
            nv_h = dpool.tile([P, 2, TT4, D], BF16, name="nv_h")
            nkT_g = dpool.tile([2, P, 2, ET, CHUNK], BF16, name="nkT_g")
            nv_g = dpool.tile([2, P, 2, TT4, D], BF16, name="nv_g")

            qT_sb = big.tile([P, ET * SQ], BF16, name="qT_sb")
            kt0_sb = earlyp.tile([P, ET, CHUNK], BF16, name="kt0_sb")
            v0_sb = earlyp.tile([P, TT4, D], BF16, name="v0_sb")
            out_acc = obig.tile([P, SQ // P, D], F32, name="out_acc")
            # fp32 column-sum accumulator for the softmax denominator,
            # one [P, 512] slab per query sb-half.
            pt_acc = obig.tile([P, 2, 512], F32, name="pt_acc")

            wq_t = [abig.tile([P, D], BF16, name=f"wq{dt}") for dt in range(DT)]
            ht_t = [abig.tile([P, SQ], BF16, name=f"ht{dt}") for dt in range(DT)]
            wk_t = [abig.tile([P, D], BF16, name=f"wk{dt}") for dt in range(DT)]
            wv_t = [abig.tile([P, D], BF16, name=f"wv{dt}") for dt in range(DT)]

            bq_sb = biasp.tile([P, ET], F32, name="bq_sb")
            bk_sb = biasp.tile([P, ET], F32, name="bk_sb")
            bv_sb = biasp.tile([P, D], F32, name="bv_sb")
            sh_sb = biasp.tile([P, 1], F32, name="sh_sb")
            nc.vector.memset(sh_sb[:], SHIFT)
            ones_sb = biasp.tile([P, 1], F32, name="ones_sb")
            nc.vector.memset(ones_sb[:], 1.0)
            nc.gpsimd.memset(pt_acc[:], 0.0)

            # ---- cold-start DMA plan: A1-critical loads ride the two fast
            # HWDGE rings at per-dt granularity (first tiles land ~1.2us in);
            # everything later in the schedule queues behind them in deadline
            # order. gpsimd (SWDGE, ~2us fixed/DMA) gets only small biases.
            nc.sync.dma_start(out=bq_sb[:], in_=bq[:])
            for dt in range(DT):
                nc.sync.dma_start(out=wq_t[dt][:],
                                  in_=wq[:, dt * D:(dt + 1) * D])
                nc.scalar.dma_start(out=ht_t[dt][:],
                                    in_=ht[:, dt * SQ:(dt + 1) * SQ])
            nc.sync.dma_start(out=kt0_sb[:], in_=kcT[0])
            nc.scalar.dma_start(out=v0_sb[:], in_=vc[0])
            for dt in range(DT):
                nc.sync.dma_start(out=wk_t[dt][:],
                                  in_=wk[:, dt * D:(dt + 1) * D])
                nc.scalar.dma_start(out=wv_t[dt][:],
                                    in_=wv[:, dt * D:(dt + 1) * D])
            nc.gpsimd.dma_start(out=bk_sb[:], in_=bk[:])
            nc.gpsimd.dma_start(out=bv_sb[:], in_=bv[:])

            # ---- Phase A1: Q^T projection, contraction split in two halves
            # so the PE starts dense work as soon as the first tiles land.
            qa_sb = abig.tile([P, ET * SQ], BF16, name="qa_sb")
            SPLIT = 4
            for et in range(ET):
                for sc in range(SQ // 512):
                    ps = spsum.tile([P, 512], F32, name="ps_q", tag="sp")
                    for dt in range(SPLIT):
                        nc.tensor.matmul(
                            ps[:],
                            wq_t[dt][:, et * P:(et + 1) * P],
                            ht_t[dt][:, sc * 512:(sc + 1) * 512],
                            start=(dt == 0), stop=(dt == SPLIT - 1))
                    nc.scalar.activation(
                        qa_sb[:, et * SQ + sc * 512:et * SQ + (sc + 1) * 512],
                        ps[:], mybir.ActivationFunctionType.Identity,
                        bias=bq_sb[:, et:et + 1])
            for et in range(ET):
                for sc in range(SQ // 512):
                    ps = spsum.tile([P, 512], F32, name="ps_q2", tag="sp")
                    for dt in range(SPLIT, DT):
                        nc.tensor.matmul(
                            ps[:],
                            wq_t[dt][:, et * P:(et + 1) * P],
                            ht_t[dt][:, sc * 512:(sc + 1) * 512],
                            start=(dt == SPLIT), stop=(dt == DT - 1))
                    nc.vector.tensor_add(
                        qT_sb[:, et * SQ + sc * 512:et * SQ + (sc + 1) * 512],
                        qa_sb[:, et * SQ + sc * 512:et * SQ + (sc + 1) * 512],
                        ps[:])

            def emit_a2():
                # new K^T -> DRAM scratch, then pair AllGather.
                for et in range(ET):
                    for sc in range(SQ // 512):
                        ps = spsum.tile([P, 512], F32, name="ps_k", tag="sp")
                        for dt in range(DT):
                            nc.tensor.matmul(
                                ps[:],
                                wk_t[dt][:, et * P:(et + 1) * P],
                                ht_t[dt][:, sc * 512:(sc + 1) * 512],
                                start=(dt == 0), stop=(dt == DT - 1))
                        st = stagep.tile([P, 512], BF16, name="st_k",
                                         tag="stage")
                        nc.scalar.activation(
                            st[:], ps[:],
                            mybir.ActivationFunctionType.Identity,
                            bias=bk_sb[:, et:et + 1])
                        q = nc.sync if et % 2 == 0 else nc.scalar
                        q.dma_start(out=nkT_h[:, sc, et, :], in_=st[:])
                nc.gpsimd.collective_compute(
                    "AllGather",
                    mybir.AluOpType.bypass,
                    replica_groups=[[0, 1], [2, 3], [4, 5], [6, 7]],
                    ins=[nkT_h[:]],
                    outs=[nkT_g[:]])

            def emit_a3():
                # new V -> DRAM scratch, then pair AllGather.
                for tt in range(SQ // P):
                    for ec in range(D // 512):
                        ps = spsum.tile([P, 512], F32, name="ps_v", tag="sp")
                        for dt in range(DT):
                            nc.tensor.matmul(
                                ps[:],
                                ht_t[dt][:, tt * P:(tt + 1) * P],
                                wv_t[dt][:, ec * 512:(ec + 1) * 512],
                                start=(dt == 0), stop=(dt == DT - 1))
                        st = stagep.tile([P, 512], BF16, name="st_v",
                                         tag="stage")
                        nc.vector.tensor_add(st[:], ps[:],
                                             bv_sb[:, ec * 512:(ec + 1) * 512])
                        q = nc.sync if (tt + ec) % 2 == 0 else nc.scalar
                        q.dma_start(
                            out=nv_h[:, tt // TT4, tt % TT4,
                                     ec * 512:(ec + 1) * 512],
                            in_=st[:])
                nc.gpsimd.collective_compute(
                    "AllGather",
                    mybir.AluOpType.bypass,
                    replica_groups=[[0, 1], [2, 3], [4, 5], [6, 7]],
                    ins=[nv_h[:]],
                    outs=[nv_g[:]])

            # denominator PSUM: 8 columns, one per (sb, si) query ptile.
            dn_q = dnpsum.tile([P, 8], F32, name="dn_q")

            # ---- Phase B: attention over 16 kv chunks, with A2/A3 emitted
            # between early chunks ----
            for c in range(N_CHUNKS):
                if c == 2:
                    emit_a2()
                elif c == 4:
                    emit_a3()
                src = CHUNK_SRC[c]
                if c == 0:
                    kt_sb, v_sb = kt0_sb, v0_sb
                else:
                    kt_sb = kpool.tile([P, ET, CHUNK], BF16, name="kt_sb")
                    v_sb = vpool.tile([P, TT4, D], BF16, name="v_sb")
                if c == 0:
                    pass
                elif src[0] == "c":
                    ci = src[1]
                    nc.sync.dma_start(out=kt_sb[:], in_=kcT[ci])
                    nc.scalar.dma_start(out=v_sb[:], in_=vc[ci])
                else:
                    rank, lc = src[1], src[2]
                    nc.sync.dma_start(out=kt_sb[:], in_=nkT_g[rank, :, lc])
                    nc.scalar.dma_start(out=v_sb[:], in_=nv_g[rank, :, lc])

                last = (c == N_CHUNKS - 1)
                for sb in range(SQ // 512):
                    pt = ptpool.tile([P, TT4, 512], BF16, name="pt")
                    for tt4 in range(TT4):
                        stp = spsum.tile([P, 512], F32, name="stp", tag="sp")
                        for et in range(ET):
                            nc.tensor.matmul(
                                stp[:],
                                kt_sb[:, et, tt4 * P:(tt4 + 1) * P],
                                qT_sb[:, et * SQ + sb * 512:
                                      et * SQ + (sb + 1) * 512],
                                start=(et == 0), stop=(et == ET - 1))
                        nc.scalar.activation(
                            pt[:, tt4, :], stp[:],
                            mybir.ActivationFunctionType.Exp,
                            bias=sh_sb[:], scale=SCALE)
                        # Pool-engine denominator accumulation (off the PE).
                        nc.gpsimd.tensor_add(pt_acc[:, sb, :],
                                             pt_acc[:, sb, :], pt[:, tt4, :])
                    rec = None
                    for si in range(4):
                        si_g = sb * 4 + si
                        if last and si == 2:
                            # pt_acc for this sb is final once its 4 Pool adds
                            # drain (hidden under PV si=0/1). Reduce across
                            # partitions with 4 tiny N=1 matmuls.
                            for sj in range(4):
                                nc.tensor.matmul(
                                    dn_q[:, sb * 4 + sj:sb * 4 + sj + 1],
                                    pt_acc[:, sb, sj * P:(sj + 1) * P],
                                    ones_sb[:, 0:1], start=True, stop=True)
                            rec = finp.tile([P, 4], F32, name="rec")
                            nc.vector.reciprocal(
                                rec[:], dn_q[:, sb * 4:sb * 4 + 4])
                        po = opsum.tile([P, D], F32, name="po")
                        for tt4 in range(TT4):
                            lhs = pt[:, tt4, si * P:(si + 1) * P]
                            st0 = (tt4 == 0)
                            sp1 = (tt4 == TT4 - 1)
                            nc.tensor.matmul(po[:, 0:512], lhs,
                                             v_sb[:, tt4, 0:512],
                                             start=st0, stop=sp1)
                            nc.tensor.matmul(po[:, 512:1024], lhs,
                                             v_sb[:, tt4, 512:1024],
                                             start=st0, stop=sp1)
                        if c == 0:
                            nc.vector.tensor_copy(out_acc[:, si_g, :], po[:])
                        else:
                            nc.vector.tensor_add(out_acc[:, si_g, :],
                                                 out_acc[:, si_g, :], po[:])
                    if last:
                        # normalize and stream out; sb=0's tail hides under
                        # sb=1's matmuls. Alternate the scale between ACT and
                        # DVE; split each store across the two HWDGE rings.
                        for si in range(4):
                            si_g = sb * 4 + si
                            ost = finp.tile([P, D], BF16, name="ost")
                            if si % 2 == 0:
                                nc.scalar.activation(
                                    ost[:], out_acc[:, si_g, :D],
                                    mybir.ActivationFunctionType.Copy,
                                    scale=rec[:, si:si + 1])
                            else:
                                nc.vector.tensor_scalar_mul(
                                    ost[:], out_acc[:, si_g, :D],
                                    rec[:, si:si + 1])
                            nc.sync.dma_start(
                                out=out[si_g * P:(si_g + 1) * P, 0:512],
                                in_=ost[:, 0:512])
                            nc.scalar.dma_start(
                                out=out[si_g * P:(si_g + 1) * P, 512:1024],
                                in_=ost[:, 512:1024])

    nc.compile()
    return nc


def _prep(hidden_states, cached_key, cached_value, Wq, bq, Wk, bk, Wv, bv):
    """Host-side resharding into SBUF-image layouts (pure reshapes/copies)."""
    def ptile_cols(a):  # [R, C] with R = n*128 -> [128, n*C] (partition-major)
        n = a.shape[0] // P
        return np.ascontiguousarray(
            a.reshape(n, P, a.shape[1]).transpose(1, 0, 2)).reshape(P, -1)

    w_h = {}
    for nm, W in (("wq", Wq), ("wk", Wk), ("wv", Wv)):
        w_h[nm] = ptile_cols(np.ascontiguousarray(W.T)).astype(NPBF16)
    bq_h = np.ascontiguousarray(bq.reshape(ET, P).T)             # [128, 8]
    bk_h = np.ascontiguousarray(bk.reshape(ET, P).T)
    bv_h = np.ascontiguousarray(np.broadcast_to(bv, (P, D)))     # [128, 1024]

    in_maps = []
    for b in range(B):
        ht_full = ptile_cols(np.ascontiguousarray(hidden_states[b].T))  # [128, 8*2048]
        # chunk-major: [chunk, P, ET, CHUNK] / [chunk, P, TT4, D]
        kcT_h = ptile_cols(np.ascontiguousarray(cached_key[b].T)) \
            .astype(NPBF16).reshape(P, ET, N_CACHED_CHUNKS, CHUNK)
        kcT_h = np.ascontiguousarray(kcT_h.transpose(2, 0, 1, 3))
        vc_h = np.ascontiguousarray(
            cached_value[b].reshape(N_CACHED_CHUNKS, TT4, P, D)
            .transpose(0, 2, 1, 3)).astype(NPBF16)
        for h in range(2):
            ht_v = ht_full.reshape(P, DT, S_NEW)
            ht_c = np.ascontiguousarray(
                ht_v[:, :, h * SQ:(h + 1) * SQ]).reshape(P, DT * SQ) \
                .astype(NPBF16)
            in_maps.append({
                "ht": ht_c, "kcT": kcT_h, "vc": vc_h,
                "wq": w_h["wq"], "wk": w_h["wk"], "wv": w_h["wv"],
                "bq": bq_h, "bk": bk_h, "bv": bv_h,
            })
    return in_maps


def kernel(hidden_states, cached_key, cached_value, Wq, bq, Wk, bk, Wv, bv,
           _trace=False):
    if "nc" not in _cache:
        _cache["nc"] = _build()
    nc = _cache["nc"]
    in_maps = _prep(
        np.asarray(hidden_states, dtype=np.float32),
        np.asarray(cached_key, dtype=np.float32),
        np.asarray(cached_value, dtype=np.float32),
        np.asarray(Wq, dtype=np.float32), np.asarray(bq, dtype=np.float32),
        np.asarray(Wk, dtype=np.float32), np.asarray(bk, dtype=np.float32),
        np.asarray(Wv, dtype=np.float32), np.asarray(bv, dtype=np.float32))
    res = run_bass_kernel_spmd(nc, in_maps, list(range(N_CORES)), trace=_trace)
    _cache["last_result"] = res
    out = np.empty((B, S_NEW, D), np.float32)
    for b in range(B):
        for h in range(2):
            out[b, h * SQ:(h + 1) * SQ, :] = \
                res.results[2 * b + h]["out"].astype(np.float32)
    return out


# revision 9
# speedup vs baseline: 1.1012x; 1.0342x over previous
"""KV-cache attention Bass kernel for Trainium2, 8 NeuronCores.

Sharding: batch (4) x query-half (2) -> 8 cores. Each core projects Q for its
1024 query rows, projects the full new K/V for its batch (duplicated across the
core pair), and runs softmax(Q K^T / 8) V over the 8192-row concatenated cache.

Layout strategy (everything kept in matmul-native layouts, no on-device
transposes):
  - scores are computed TRANSPOSED: S^T[t, s] with lhsT = K^T tile, rhs = Q^T.
  - softmax over t (partition dim) uses exp with a constant shift (exact:
    softmax is shift-invariant); the denominator is accumulated on the Pool
    engine (pt_acc += exp tile, fp32) and reduced across partitions at the
    very end with 8 tiny N=1 matmuls -> per-partition reciprocal scale.
  - P^T is exactly the stationary operand layout the PV matmul needs, so no
    transposes are ever required.

Phase schedule (PE never idles on DMA/collectives):
  A1 (Q^T) | B0 B1 | A2 (new K^T -> DRAM, pair AllGather) | B2 B3 |
  A3 (new V -> DRAM, pair AllGather) | B4..B11 | new-KV chunks 12..15 | tail.

DMA plan. DMA-issue ops occupy the issuing engine's instruction FIFO and
stall on ring backpressure, so:
  - sync (HWDGE, otherwise idle): bq + wq dt0-3 fine-grained (A1 starts
    ~1.5us after the first pair lands), in-loop K^T chunk loads, all output
    stores (keeps the tail off the ACT queue).
  - scalar (HWDGE, runs ACT): only ht dt0-3 before its first activation,
    then in-loop V chunk loads (~0.6us issue each, ring never backlogged).
  - gpsimd (SWDGE, 341 GB/s at >=1MB): the bulk - wq/ht dt4-7, kt0, v0,
    wk, wv as 1-2MB transfers, then tiny bk/bv.
"""
import sys
import numpy as np

if "/opt/trn_rl_repo" not in sys.path:
    sys.path.insert(0, "/opt/trn_rl_repo")

import ml_dtypes
import concourse.bacc as bacc
import concourse.mybir as mybir
from concourse.tile import TileContext
from concourse.bass_utils import run_bass_kernel_spmd

B, S_NEW, S_CACHE, D = 4, 2048, 6144, 1024
S_KV = S_CACHE + S_NEW            # 8192
SQ = S_NEW // 2                   # 1024 query rows per core
N_CORES = 8
P = 128
ET = D // P                       # 8 feature tiles
DT = D // P                       # 8 contraction tiles
HDT = DT // 2                     # 4: contraction tiles per A1 half
CHUNK = 512                       # kv rows per chunk
N_CHUNKS = S_KV // CHUNK          # 16 (12 cached + 4 new)
N_CACHED_CHUNKS = S_CACHE // CHUNK
TT4 = CHUNK // P                  # 4 t-ptiles per chunk
SCALE = 0.125                     # 1/sqrt(64)
SHIFT = -16.0                     # constant softmax shift (exact)

F32 = mybir.dt.float32
BF16 = mybir.dt.bfloat16
NPBF16 = np.dtype(ml_dtypes.bfloat16)

# chunk processing order: cached chunks first, AllGather-dependent new-KV
# chunks at the very end (the gathers complete ~200us before they are needed).
CHUNK_SRC = (
    [("c", i) for i in range(N_CACHED_CHUNKS)]
    + [("n", r, l) for r in range(2) for l in range(2)]
)

_cache = {}


def _build():
    nc = bacc.Bacc("TRN2", target_bir_lowering=False, debug=False,
                   num_devices=N_CORES)
    ht = nc.dram_tensor("ht", [P, DT * SQ], BF16, kind="ExternalInput")
    wq = nc.dram_tensor("wq", [P, DT * D], BF16, kind="ExternalInput")
    wk = nc.dram_tensor("wk", [P, DT * D], BF16, kind="ExternalInput")
    wv = nc.dram_tensor("wv", [P, DT * D], BF16, kind="ExternalInput")
    # chunk-major cache layouts: one chunk's K^T/V is fully contiguous per
    # partition (8KB lines) so each phase-B load runs at full DMA efficiency.
    kcT = nc.dram_tensor("kcT", [N_CACHED_CHUNKS, P, ET, CHUNK], BF16,
                         kind="ExternalInput")
    vc = nc.dram_tensor("vc", [N_CACHED_CHUNKS, P, TT4, D], BF16,
                        kind="ExternalInput")
    bq = nc.dram_tensor("bq", [P, ET], F32, kind="ExternalInput")
    bk = nc.dram_tensor("bk", [P, ET], F32, kind="ExternalInput")
    bv = nc.dram_tensor("bv", [P, D], F32, kind="ExternalInput")
    out = nc.dram_tensor("out", [SQ, D], BF16, kind="ExternalOutput")

    with TileContext(nc) as tc:
        with tc.tile_pool(name="big", bufs=1) as big, \
             tc.tile_pool(name="bias", bufs=1) as biasp, \
             tc.tile_pool(name="spsum", bufs=3, space="PSUM") as spsum, \
             tc.tile_pool(name="dnpsum", bufs=1, space="PSUM") as dnpsum, \
             tc.tile_pool(name="opsum", bufs=2, space="PSUM") as opsum, \
             tc.tile_pool(name="early", bufs=1) as earlyp, \
             tc.tile_pool(name="abig", bufs=1) as abig, \
             tc.tile_pool(name="stage", bufs=4) as stagep, \
             tc.tile_pool(name="kpool", bufs=2) as kpool, \
             tc.tile_pool(name="vpool", bufs=2) as vpool, \
             tc.tile_pool(name="ptpool", bufs=2) as ptpool, \
             tc.tile_pool(name="fin", bufs=4) as finp, \
             tc.tile_pool(name="obig", bufs=1) as obig, \
             tc.tile_pool(name="dram", bufs=1, space="DRAM") as dpool:

            # new-KV scratch, local-chunk-major so gathered loads are
            # contiguous per partition as well.
            nkT_h = dpool.tile([P, 2, ET, CHUNK], BF16, name="nkT_h")
            nv_h = dpool.tile([P, 2, TT4, D], BF16, name="nv_h")
            nkT_g = dpool.tile([2, P, 2, ET, CHUNK], BF16, name="nkT_g")
            nv_g = dpool.tile([2, P, 2, TT4, D], BF16, name="nv_g")

            qT_sb = big.tile([P, ET * SQ], BF16, name="qT_sb")
            kt0_sb = earlyp.tile([P, ET, CHUNK], BF16, name="kt0_sb")
            v0_sb = earlyp.tile([P, TT4, D], BF16, name="v0_sb")
            out_acc = obig.tile([P, SQ // P, D], F32, name="out_acc")
            # fp32 column-sum accumulator for the softmax denominator,
            # one [P, 512] slab per query sb-half.
            pt_acc = obig.tile([P, 2, 512], F32, name="pt_acc")

            # wq/ht split in dt-halves: dt0-3 fine-grained on the HWDGE
            # rings, dt4-7 as single 1MB SWDGE transfers.
            wqA = abig.tile([P, HDT * D], BF16, name="wqA")
            wqB = abig.tile([P, HDT * D], BF16, name="wqB")
            htA = abig.tile([P, HDT * SQ], BF16, name="htA")
            htB = abig.tile([P, HDT * SQ], BF16, name="htB")
            wk_sb = abig.tile([P, DT * D], BF16, name="wk_sb")
            wv_sb = abig.tile([P, DT * D], BF16, name="wv_sb")

            def wq_ap(dt, lo, hi):
                t, d = (wqA, dt) if dt < HDT else (wqB, dt - HDT)
                return t[:, d * D + lo:d * D + hi]

            def ht_ap(dt, lo, hi):
                t, d = (htA, dt) if dt < HDT else (htB, dt - HDT)
                return t[:, d * SQ + lo:d * SQ + hi]

            bq_sb = biasp.tile([P, ET], F32, name="bq_sb")
            bk_sb = biasp.tile([P, ET], F32, name="bk_sb")
            bv_sb = biasp.tile([P, D], F32, name="bv_sb")
            sh_sb = biasp.tile([P, 1], F32, name="sh_sb")
            nc.vector.memset(sh_sb[:], SHIFT)
            ones_sb = biasp.tile([P, 1], F32, name="ones_sb")
            nc.vector.memset(ones_sb[:], 1.0)
            nc.vector.memset(pt_acc[:], 0.0)

            # All early loads ride the single SWDGE queue in strict deadline
            # order (HBM stays dedicated to the critical path): dt0 pair
            # fine-grained so A1 starts ~10.5us in, then progressively
            # coarser. The HWDGE rings stay empty until steady state; chunk-1
            # KV is hoisted here too so nothing else touches HBM early.
            nc.sync.dma_start(out=bq_sb[:], in_=bq[:])
            nc.gpsimd.dma_start(out=wqA[:, 0:D], in_=wq[:, 0:D])
            nc.gpsimd.dma_start(out=htA[:, 0:SQ], in_=ht[:, 0:SQ])
            nc.gpsimd.dma_start(out=wqA[:, D:HDT * D], in_=wq[:, D:HDT * D])
            nc.gpsimd.dma_start(out=htA[:, SQ:HDT * SQ],
                                in_=ht[:, SQ:HDT * SQ])
            nc.gpsimd.dma_start(out=wqB[:], in_=wq[:, HDT * D:])
            nc.gpsimd.dma_start(out=htB[:], in_=ht[:, HDT * SQ:])
            nc.gpsimd.dma_start(out=kt0_sb[:], in_=kcT[0])
            nc.gpsimd.dma_start(out=v0_sb[:], in_=vc[0])
            kt1_sb = kpool.tile([P, ET, CHUNK], BF16, name="kt_sb")
            v1_sb = vpool.tile([P, TT4, D], BF16, name="v_sb")
            nc.gpsimd.dma_start(out=kt1_sb[:], in_=kcT[1])
            nc.gpsimd.dma_start(out=v1_sb[:], in_=vc[1])
            nc.gpsimd.dma_start(out=wk_sb[:], in_=wk[:])
            nc.gpsimd.dma_start(out=wv_sb[:], in_=wv[:])
            nc.gpsimd.dma_start(out=bk_sb[:], in_=bk[:])
            nc.gpsimd.dma_start(out=bv_sb[:], in_=bv[:])

            # ---- Phase A1: Q^T projection, contraction split in two halves
            # so the PE starts dense work as soon as the first tiles land.
            qa_sb = abig.tile([P, ET * SQ], BF16, name="qa_sb")
            for et in range(ET):
                for sc in range(SQ // 512):
                    ps = spsum.tile([P, 512], F32, name="ps_q", tag="sp")
                    for dt in range(HDT):
                        nc.tensor.matmul(
                            ps[:],
                            wq_ap(dt, et * P, (et + 1) * P),
                            ht_ap(dt, sc * 512, (sc + 1) * 512),
                            start=(dt == 0), stop=(dt == HDT - 1))
                    nc.scalar.activation(
                        qa_sb[:, et * SQ + sc * 512:et * SQ + (sc + 1) * 512],
                        ps[:], mybir.ActivationFunctionType.Identity,
                        bias=bq_sb[:, et:et + 1])
            for et in range(ET):
                for sc in range(SQ // 512):
                    ps = spsum.tile([P, 512], F32, name="ps_q2", tag="sp")
                    for dt in range(HDT, DT):
                        nc.tensor.matmul(
                            ps[:],
                            wq_ap(dt, et * P, (et + 1) * P),
                            ht_ap(dt, sc * 512, (sc + 1) * 512),
                            start=(dt == HDT), stop=(dt == DT - 1))
                    nc.vector.tensor_add(
                        qT_sb[:, et * SQ + sc * 512:et * SQ + (sc + 1) * 512],
                        qa_sb[:, et * SQ + sc * 512:et * SQ + (sc + 1) * 512],
                        ps[:])

            def emit_a2():
                # new K^T -> DRAM scratch, then pair AllGather.
                for et in range(ET):
                    for sc in range(SQ // 512):
                        ps = spsum.tile([P, 512], F32, name="ps_k", tag="sp")
                        for dt in range(DT):
                            nc.tensor.matmul(
                                ps[:],
                                wk_sb[:, dt * D + et * P:dt * D + (et + 1) * P],
                                ht_ap(dt, sc * 512, (sc + 1) * 512),
                                start=(dt == 0), stop=(dt == DT - 1))
                        st = stagep.tile([P, 512], BF16, name="st_k",
                                         tag="stage")
                        nc.scalar.activation(
                            st[:], ps[:],
                            mybir.ActivationFunctionType.Identity,
                            bias=bk_sb[:, et:et + 1])
                        q = nc.sync if et % 2 == 0 else nc.scalar
                        q.dma_start(out=nkT_h[:, sc, et, :], in_=st[:])
                nc.gpsimd.collective_compute(
                    "AllGather",
                    mybir.AluOpType.bypass,
                    replica_groups=[[0, 1], [2, 3], [4, 5], [6, 7]],
                    ins=[nkT_h[:]],
                    outs=[nkT_g[:]])

            def emit_a3():
                # new V -> DRAM scratch, then pair AllGather.
                for tt in range(SQ // P):
                    for ec in range(D // 512):
                        ps = spsum.tile([P, 512], F32, name="ps_v", tag="sp")
                        for dt in range(DT):
                            nc.tensor.matmul(
                                ps[:],
                                ht_ap(dt, tt * P, (tt + 1) * P),
                                wv_sb[:, dt * D + ec * 512:
                                      dt * D + (ec + 1) * 512],
                                start=(dt == 0), stop=(dt == DT - 1))
                        st = stagep.tile([P, 512], BF16, name="st_v",
                                         tag="stage")
                        nc.vector.tensor_add(st[:], ps[:],
                                             bv_sb[:, ec * 512:(ec + 1) * 512])
                        q = nc.sync if (tt + ec) % 2 == 0 else nc.scalar
                        q.dma_start(
                            out=nv_h[:, tt // TT4, tt % TT4,
                                     ec * 512:(ec + 1) * 512],
                            in_=st[:])
                nc.gpsimd.collective_compute(
                    "AllGather",
                    mybir.AluOpType.bypass,
                    replica_groups=[[0, 1], [2, 3], [4, 5], [6, 7]],
                    ins=[nv_h[:]],
                    outs=[nv_g[:]])

            # denominator PSUM: 8 columns, one per (sb, si) query ptile.
            dn_q = dnpsum.tile([P, 8], F32, name="dn_q")

            # ---- Phase B: attention over 16 kv chunks, with A2/A3 emitted
            # between early chunks ----
            for c in range(N_CHUNKS):
                if c == 2:
                    emit_a2()
                elif c == 4:
                    emit_a3()
                src = CHUNK_SRC[c]
                if c == 0:
                    kt_sb, v_sb = kt0_sb, v0_sb
                elif c == 1:
                    kt_sb, v_sb = kt1_sb, v1_sb
                else:
                    kt_sb = kpool.tile([P, ET, CHUNK], BF16, name="kt_sb")
                    v_sb = vpool.tile([P, TT4, D], BF16, name="v_sb")
                if c <= 1:
                    pass
                elif src[0] == "c":
                    ci = src[1]
                    nc.sync.dma_start(out=kt_sb[:], in_=kcT[ci])
                    nc.scalar.dma_start(out=v_sb[:], in_=vc[ci])
                else:
                    rank, lc = src[1], src[2]
                    nc.sync.dma_start(out=kt_sb[:], in_=nkT_g[rank, :, lc])
                    nc.scalar.dma_start(out=v_sb[:], in_=nv_g[rank, :, lc])

                last = (c == N_CHUNKS - 1)
                for sb in range(SQ // 512):
                    pt = ptpool.tile([P, TT4, 512], BF16, name="pt")
                    for tt4 in range(TT4):
                        stp = spsum.tile([P, 512], F32, name="stp", tag="sp")
                        for et in range(ET):
                            nc.tensor.matmul(
                                stp[:],
                                kt_sb[:, et, tt4 * P:(tt4 + 1) * P],
                                qT_sb[:, et * SQ + sb * 512:
                                      et * SQ + (sb + 1) * 512],
                                start=(et == 0), stop=(et == ET - 1))
                        nc.scalar.activation(
                            pt[:, tt4, :], stp[:],
                            mybir.ActivationFunctionType.Exp,
                            bias=sh_sb[:], scale=SCALE)
                        # Pool-engine denominator accumulation (off the PE).
                        nc.gpsimd.tensor_add(pt_acc[:, sb, :],
                                             pt_acc[:, sb, :], pt[:, tt4, :])
                    rec = None
                    for si in range(4):
                        si_g = sb * 4 + si
                        if last and si == 2:
                            # pt_acc for this sb is final once its 4 Pool adds
                            # drain (hidden under PV si=0/1). Reduce across
                            # partitions with 4 tiny N=1 matmuls.
                            for sj in range(4):
                                nc.tensor.matmul(
                                    dn_q[:, sb * 4 + sj:sb * 4 + sj + 1],
                                    pt_acc[:, sb, sj * P:(sj + 1) * P],
                                    ones_sb[:, 0:1], start=True, stop=True)
                            rec = finp.tile([P, 4], F32, name="rec")
                            nc.vector.reciprocal(
                                rec[:], dn_q[:, sb * 4:sb * 4 + 4])
                        po = opsum.tile([P, D], F32, name="po")
                        for tt4 in range(TT4):
                            lhs = pt[:, tt4, si * P:(si + 1) * P]
                            st0 = (tt4 == 0)
                            sp1 = (tt4 == TT4 - 1)
                            nc.tensor.matmul(po[:, 0:512], lhs,
                                             v_sb[:, tt4, 0:512],
                                             start=st0, stop=sp1)
                            nc.tensor.matmul(po[:, 512:1024], lhs,
                                             v_sb[:, tt4, 512:1024],
                                             start=st0, stop=sp1)
                        if c == 0:
                            nc.vector.tensor_copy(out_acc[:, si_g, :], po[:])
                        else:
                            nc.vector.tensor_add(out_acc[:, si_g, :],
                                                 out_acc[:, si_g, :], po[:])
                    if last:
                        # normalize and stream out; sb=0's tail hides under
                        # sb=1's matmuls. sb=0 scales ride the DVE so the ACT
                        # queue stays clear for sb=1's exps; all stores go to
                        # the otherwise-idle sync ring.
                        for si in range(4):
                            si_g = sb * 4 + si
                            ost = finp.tile([P, D], BF16, name="ost")
                            if sb == 1 and si % 2 == 0:
                                nc.scalar.activation(
                                    ost[:], out_acc[:, si_g, :D],
                                    mybir.ActivationFunctionType.Copy,
                                    scale=rec[:, si:si + 1])
                            else:
                                nc.vector.tensor_scalar_mul(
                                    ost[:], out_acc[:, si_g, :D],
                                    rec[:, si:si + 1])
                            nc.sync.dma_start(
                                out=out[si_g * P:(si_g + 1) * P, :],
                                in_=ost[:])

    nc.compile()
    return nc


def _prep(hidden_states, cached_key, cached_value, Wq, bq, Wk, bk, Wv, bv):
    """Host-side resharding into SBUF-image layouts (pure reshapes/copies)."""
    def ptile_cols(a):  # [R, C] with R = n*128 -> [128, n*C] (partition-major)
        n = a.shape[0] // P
        return np.ascontiguousarray(
            a.reshape(n, P, a.shape[1]).transpose(1, 0, 2)).reshape(P, -1)

    w_h = {}
    for nm, W in (("wq", Wq), ("wk", Wk), ("wv", Wv)):
        w_h[nm] = ptile_cols(np.ascontiguousarray(W.T)).astype(NPBF16)
    bq_h = np.ascontiguousarray(bq.reshape(ET, P).T)             # [128, 8]
    bk_h = np.ascontiguousarray(bk.reshape(ET, P).T)
    bv_h = np.ascontiguousarray(np.broadcast_to(bv, (P, D)))     # [128, 1024]

    in_maps = []
    for b in range(B):
        ht_full = ptile_cols(np.ascontiguousarray(hidden_states[b].T))  # [128, 8*2048]
        # chunk-major: [chunk, P, ET, CHUNK] / [chunk, P, TT4, D]
        kcT_h = ptile_cols(np.ascontiguousarray(cached_key[b].T)) \
            .astype(NPBF16).reshape(P, ET, N_CACHED_CHUNKS, CHUNK)
        kcT_h = np.ascontiguousarray(kcT_h.transpose(2, 0, 1, 3))
        vc_h = np.ascontiguousarray(
            cached_value[b].reshape(N_CACHED_CHUNKS, TT4, P, D)
            .transpose(0, 2, 1, 3)).astype(NPBF16)
        for h in range(2):
            ht_v = ht_full.reshape(P, DT, S_NEW)
            ht_c = np.ascontiguousarray(
                ht_v[:, :, h * SQ:(h + 1) * SQ]).reshape(P, DT * SQ) \
                .astype(NPBF16)
            in_maps.append({
                "ht": ht_c, "kcT": kcT_h, "vc": vc_h,
                "wq": w_h["wq"], "wk": w_h["wk"], "wv": w_h["wv"],
                "bq": bq_h, "bk": bk_h, "bv": bv_h,
            })
    return in_maps


def kernel(hidden_states, cached_key, cached_value, Wq, bq, Wk, bk, Wv, bv,
           _trace=False):
    if "nc" not in _cache:
        _cache["nc"] = _build()
    nc = _cache["nc"]
    in_maps = _prep(
        np.asarray(hidden_states, dtype=np.float32),
        np.asarray(cached_key, dtype=np.float32),
        np.asarray(cached_value, dtype=np.float32),
        np.asarray(Wq, dtype=np.float32), np.asarray(bq, dtype=np.float32),
        np.asarray(Wk, dtype=np.float32), np.asarray(bk, dtype=np.float32),
        np.asarray(Wv, dtype=np.float32), np.asarray(bv, dtype=np.float32))
    res = run_bass_kernel_spmd(nc, in_maps, list(range(N_CORES)), trace=_trace)
    _cache["last_result"] = res
    out = np.empty((B, S_NEW, D), np.float32)
    for b in range(B):
        for h in range(2):
            out[b, h * SQ:(h + 1) * SQ, :] = \
                res.results[2 * b + h]["out"].astype(np.float32)
    return out


# revision 10
# speedup vs baseline: 1.1083x; 1.0065x over previous
"""KV-cache attention Bass kernel for Trainium2, 8 NeuronCores.

Sharding: batch (4) x query-half (2) -> 8 cores. Each core projects Q for its
1024 query rows, projects the full new K/V for its batch (duplicated across the
core pair), and runs softmax(Q K^T / 8) V over the 8192-row concatenated cache.

Layout strategy (everything kept in matmul-native layouts, no on-device
transposes):
  - scores are computed TRANSPOSED: S^T[t, s] with lhsT = K^T tile, rhs = Q^T.
  - softmax over t (partition dim) uses exp with a constant shift (exact:
    softmax is shift-invariant); the denominator is accumulated on the Pool
    engine (pt_acc += exp tile, fp32) and reduced across partitions at the
    very end with 8 tiny N=1 matmuls -> per-partition reciprocal scale.
  - P^T is exactly the stationary operand layout the PV matmul needs, so no
    transposes are ever required.

Phase schedule (PE never idles on DMA/collectives):
  A1 (Q^T) | B0 B1 | A2 (new K^T -> DRAM, pair AllGather) | B2 B3 |
  A3 (new V -> DRAM, pair AllGather) | B4..B11 | new-KV chunks 12..15 | tail.

DMA plan. DMA-issue ops occupy the issuing engine's instruction FIFO and
stall on ring backpressure, so:
  - sync (HWDGE, otherwise idle): bq + wq dt0-3 fine-grained (A1 starts
    ~1.5us after the first pair lands), in-loop K^T chunk loads, all output
    stores (keeps the tail off the ACT queue).
  - scalar (HWDGE, runs ACT): only ht dt0-3 before its first activation,
    then in-loop V chunk loads (~0.6us issue each, ring never backlogged).
  - gpsimd (SWDGE, 341 GB/s at >=1MB): the bulk - wq/ht dt4-7, kt0, v0,
    wk, wv as 1-2MB transfers, then tiny bk/bv.
"""
import sys
import numpy as np

if "/opt/trn_rl_repo" not in sys.path:
    sys.path.insert(0, "/opt/trn_rl_repo")

import ml_dtypes
import concourse.bacc as bacc
import concourse.mybir as mybir
from concourse.tile import TileContext
from concourse.bass_utils import run_bass_kernel_spmd

B, S_NEW, S_CACHE, D = 4, 2048, 6144, 1024
S_KV = S_CACHE + S_NEW            # 8192
SQ = S_NEW // 2                   # 1024 query rows per core
N_CORES = 8
P = 128
ET = D // P                       # 8 feature tiles
DT = D // P                       # 8 contraction tiles
HDT = DT // 2                     # 4: contraction tiles per A1 half
CHUNK = 512                       # kv rows per chunk
N_CHUNKS = S_KV // CHUNK          # 16 (12 cached + 4 new)
N_CACHED_CHUNKS = S_CACHE // CHUNK
TT4 = CHUNK // P                  # 4 t-ptiles per chunk
SCALE = 0.125                     # 1/sqrt(64)
SHIFT = -16.0                     # constant softmax shift (exact)

F32 = mybir.dt.float32
BF16 = mybir.dt.bfloat16
NPBF16 = np.dtype(ml_dtypes.bfloat16)

# chunk processing order: cached chunks first, AllGather-dependent new-KV
# chunks at the very end (the gathers complete ~200us before they are needed).
CHUNK_SRC = (
    [("c", i) for i in range(N_CACHED_CHUNKS)]
    + [("n", r, l) for r in range(2) for l in range(2)]
)

_cache = {}


def _build():
    nc = bacc.Bacc("TRN2", target_bir_lowering=False, debug=False,
                   num_devices=N_CORES)
    ht = nc.dram_tensor("ht", [P, DT * SQ], BF16, kind="ExternalInput")
    wq = nc.dram_tensor("wq", [P, DT * D], BF16, kind="ExternalInput")
    wk = nc.dram_tensor("wk", [P, DT * D], BF16, kind="ExternalInput")
    wv = nc.dram_tensor("wv", [P, DT * D], BF16, kind="ExternalInput")
    # chunk-major cache layouts: one chunk's K^T/V is fully contiguous per
    # partition (8KB lines) so each phase-B load runs at full DMA efficiency.
    kcT = nc.dram_tensor("kcT", [N_CACHED_CHUNKS, P, ET, CHUNK], BF16,
                         kind="ExternalInput")
    vc = nc.dram_tensor("vc", [N_CACHED_CHUNKS, P, TT4, D], BF16,
                        kind="ExternalInput")
    bq = nc.dram_tensor("bq", [P, ET], F32, kind="ExternalInput")
    bk = nc.dram_tensor("bk", [P, ET], F32, kind="ExternalInput")
    bv = nc.dram_tensor("bv", [P, D], F32, kind="ExternalInput")
    out = nc.dram_tensor("out", [SQ, D], BF16, kind="ExternalOutput")

    with TileContext(nc) as tc:
        with tc.tile_pool(name="big", bufs=1) as big, \
             tc.tile_pool(name="bias", bufs=1) as biasp, \
             tc.tile_pool(name="spsum", bufs=3, space="PSUM") as spsum, \
             tc.tile_pool(name="dnpsum", bufs=1, space="PSUM") as dnpsum, \
             tc.tile_pool(name="opsum", bufs=2, space="PSUM") as opsum, \
             tc.tile_pool(name="early", bufs=1) as earlyp, \
             tc.tile_pool(name="abig", bufs=1) as abig, \
             tc.tile_pool(name="stage", bufs=4) as stagep, \
             tc.tile_pool(name="kpool", bufs=2) as kpool, \
             tc.tile_pool(name="vpool", bufs=2) as vpool, \
             tc.tile_pool(name="ptpool", bufs=2) as ptpool, \
             tc.tile_pool(name="fin", bufs=4) as finp, \
             tc.tile_pool(name="obig", bufs=1) as obig, \
             tc.tile_pool(name="dram", bufs=1, space="DRAM") as dpool:

            # new-KV scratch, local-chunk-major so gathered loads are
            # contiguous per partition as well.
            nkT_h = dpool.tile([P, 2, ET, CHUNK], BF16, name="nkT_h")
            nv_h = dpool.tile([P, 2, TT4, D], BF16, name="nv_h")
            nkT_g = dpool.tile([2, P, 2, ET, CHUNK], BF16, name="nkT_g")
            nv_g = dpool.tile([2, P, 2, TT4, D], BF16, name="nv_g")

            qT_sb = big.tile([P, ET * SQ], BF16, name="qT_sb")
            kt0_sb = earlyp.tile([P, ET, CHUNK], BF16, name="kt0_sb")
            v0_sb = earlyp.tile([P, TT4, D], BF16, name="v0_sb")
            out_acc = obig.tile([P, SQ // P, D], F32, name="out_acc")
            # fp32 column-sum accumulator for the softmax denominator,
            # one [P, 512] slab per query sb-half.
            pt_acc = obig.tile([P, 2, 512], F32, name="pt_acc")

            # wq/ht split in dt-halves: dt0-3 fine-grained on the HWDGE
            # rings, dt4-7 as single 1MB SWDGE transfers.
            wqA = abig.tile([P, HDT * D], BF16, name="wqA")
            wqB = abig.tile([P, HDT * D], BF16, name="wqB")
            htA = abig.tile([P, HDT * SQ], BF16, name="htA")
            htB = abig.tile([P, HDT * SQ], BF16, name="htB")
            wk_sb = abig.tile([P, DT * D], BF16, name="wk_sb")
            wv_sb = abig.tile([P, DT * D], BF16, name="wv_sb")

            def wq_ap(dt, lo, hi):
                t, d = (wqA, dt) if dt < HDT else (wqB, dt - HDT)
                return t[:, d * D + lo:d * D + hi]

            def ht_ap(dt, lo, hi):
                t, d = (htA, dt) if dt < HDT else (htB, dt - HDT)
                return t[:, d * SQ + lo:d * SQ + hi]

            bq_sb = biasp.tile([P, ET], F32, name="bq_sb")
            bk_sb = biasp.tile([P, ET], F32, name="bk_sb")
            bv_sb = biasp.tile([P, D], F32, name="bv_sb")
            sh_sb = biasp.tile([P, 1], F32, name="sh_sb")
            nc.vector.memset(sh_sb[:], SHIFT)
            ones_sb = biasp.tile([P, 1], F32, name="ones_sb")
            nc.vector.memset(ones_sb[:], 1.0)
            nc.vector.memset(pt_acc[:], 0.0)

            # All early loads ride the single SWDGE queue in strict deadline
            # order (HBM stays dedicated to the critical path): dt0 pair
            # fine-grained so A1 starts ~10.5us in, then progressively
            # coarser. The HWDGE rings stay empty until steady state; chunk-1
            # KV is hoisted here too so nothing else touches HBM early.
            nc.sync.dma_start(out=bq_sb[:], in_=bq[:])
            nc.sync.dma_start(out=wqA[:, 0:D], in_=wq[:, 0:D])
            nc.scalar.dma_start(out=htA[:, 0:SQ], in_=ht[:, 0:SQ])
            nc.gpsimd.dma_start(out=wqA[:, D:HDT * D], in_=wq[:, D:HDT * D])
            nc.gpsimd.dma_start(out=htA[:, SQ:HDT * SQ],
                                in_=ht[:, SQ:HDT * SQ])
            nc.gpsimd.dma_start(out=wqB[:], in_=wq[:, HDT * D:])
            nc.gpsimd.dma_start(out=htB[:], in_=ht[:, HDT * SQ:])
            nc.gpsimd.dma_start(out=kt0_sb[:], in_=kcT[0])
            nc.gpsimd.dma_start(out=v0_sb[:], in_=vc[0])
            kt1_sb = kpool.tile([P, ET, CHUNK], BF16, name="kt_sb")
            v1_sb = vpool.tile([P, TT4, D], BF16, name="v_sb")
            nc.gpsimd.dma_start(out=kt1_sb[:], in_=kcT[1])
            nc.gpsimd.dma_start(out=v1_sb[:], in_=vc[1])
            nc.gpsimd.dma_start(out=wk_sb[:], in_=wk[:])
            nc.gpsimd.dma_start(out=wv_sb[:], in_=wv[:])
            nc.gpsimd.dma_start(out=bk_sb[:], in_=bk[:])
            nc.gpsimd.dma_start(out=bv_sb[:], in_=bv[:])

            # ---- Phase A1: Q^T projection, contraction split in two halves
            # so the PE starts dense work as soon as the first tiles land.
            qa_sb = abig.tile([P, ET * SQ], BF16, name="qa_sb")
            for et in range(ET):
                for sc in range(SQ // 512):
                    ps = spsum.tile([P, 512], F32, name="ps_q", tag="sp")
                    for dt in range(HDT):
                        nc.tensor.matmul(
                            ps[:],
                            wq_ap(dt, et * P, (et + 1) * P),
                            ht_ap(dt, sc * 512, (sc + 1) * 512),
                            start=(dt == 0), stop=(dt == HDT - 1))
                    nc.scalar.activation(
                        qa_sb[:, et * SQ + sc * 512:et * SQ + (sc + 1) * 512],
                        ps[:], mybir.ActivationFunctionType.Identity,
                        bias=bq_sb[:, et:et + 1])
            for et in range(ET):
                for sc in range(SQ // 512):
                    ps = spsum.tile([P, 512], F32, name="ps_q2", tag="sp")
                    for dt in range(HDT, DT):
                        nc.tensor.matmul(
                            ps[:],
                            wq_ap(dt, et * P, (et + 1) * P),
                            ht_ap(dt, sc * 512, (sc + 1) * 512),
                            start=(dt == HDT), stop=(dt == DT - 1))
                    nc.vector.tensor_add(
                        qT_sb[:, et * SQ + sc * 512:et * SQ + (sc + 1) * 512],
                        qa_sb[:, et * SQ + sc * 512:et * SQ + (sc + 1) * 512],
                        ps[:])

            def emit_a2():
                # new K^T -> DRAM scratch, then pair AllGather.
                for et in range(ET):
                    for sc in range(SQ // 512):
                        ps = spsum.tile([P, 512], F32, name="ps_k", tag="sp")
                        for dt in range(DT):
                            nc.tensor.matmul(
                                ps[:],
                                wk_sb[:, dt * D + et * P:dt * D + (et + 1) * P],
                                ht_ap(dt, sc * 512, (sc + 1) * 512),
                                start=(dt == 0), stop=(dt == DT - 1))
                        st = stagep.tile([P, 512], BF16, name="st_k",
                                         tag="stage")
                        nc.scalar.activation(
                            st[:], ps[:],
                            mybir.ActivationFunctionType.Identity,
                            bias=bk_sb[:, et:et + 1])
                        q = nc.sync if et % 2 == 0 else nc.scalar
                        q.dma_start(out=nkT_h[:, sc, et, :], in_=st[:])
                nc.gpsimd.collective_compute(
                    "AllGather",
                    mybir.AluOpType.bypass,
                    replica_groups=[[0, 1], [2, 3], [4, 5], [6, 7]],
                    ins=[nkT_h[:]],
                    outs=[nkT_g[:]])

            def emit_a3():
                # new V -> DRAM scratch, then pair AllGather.
                for tt in range(SQ // P):
                    for ec in range(D // 512):
                        ps = spsum.tile([P, 512], F32, name="ps_v", tag="sp")
                        for dt in range(DT):
                            nc.tensor.matmul(
                                ps[:],
                                ht_ap(dt, tt * P, (tt + 1) * P),
                                wv_sb[:, dt * D + ec * 512:
                                      dt * D + (ec + 1) * 512],
                                start=(dt == 0), stop=(dt == DT - 1))
                        st = stagep.tile([P, 512], BF16, name="st_v",
                                         tag="stage")
                        nc.vector.tensor_add(st[:], ps[:],
                                             bv_sb[:, ec * 512:(ec + 1) * 512])
                        q = nc.sync if (tt + ec) % 2 == 0 else nc.scalar
                        q.dma_start(
                            out=nv_h[:, tt // TT4, tt % TT4,
                                     ec * 512:(ec + 1) * 512],
                            in_=st[:])
                nc.gpsimd.collective_compute(
                    "AllGather",
                    mybir.AluOpType.bypass,
                    replica_groups=[[0, 1], [2, 3], [4, 5], [6, 7]],
                    ins=[nv_h[:]],
                    outs=[nv_g[:]])

            # denominator PSUM: 8 columns, one per (sb, si) query ptile.
            dn_q = dnpsum.tile([P, 8], F32, name="dn_q")

            # ---- Phase B: attention over 16 kv chunks, with A2/A3 emitted
            # between early chunks ----
            for c in range(N_CHUNKS):
                if c == 2:
                    emit_a2()
                elif c == 4:
                    emit_a3()
                src = CHUNK_SRC[c]
                if c == 0:
                    kt_sb, v_sb = kt0_sb, v0_sb
                elif c == 1:
                    kt_sb, v_sb = kt1_sb, v1_sb
                else:
                    kt_sb = kpool.tile([P, ET, CHUNK], BF16, name="kt_sb")
                    v_sb = vpool.tile([P, TT4, D], BF16, name="v_sb")
                if c <= 1:
                    pass
                elif src[0] == "c":
                    ci = src[1]
                    nc.sync.dma_start(out=kt_sb[:], in_=kcT[ci])
                    nc.scalar.dma_start(out=v_sb[:], in_=vc[ci])
                else:
                    rank, lc = src[1], src[2]
                    nc.sync.dma_start(out=kt_sb[:], in_=nkT_g[rank, :, lc])
                    nc.scalar.dma_start(out=v_sb[:], in_=nv_g[rank, :, lc])

                last = (c == N_CHUNKS - 1)
                for sb in range(SQ // 512):
                    pt = ptpool.tile([P, TT4, 512], BF16, name="pt")
                    for tt4 in range(TT4):
                        stp = spsum.tile([P, 512], F32, name="stp", tag="sp")
                        for et in range(ET):
                            nc.tensor.matmul(
                                stp[:],
                                kt_sb[:, et, tt4 * P:(tt4 + 1) * P],
                                qT_sb[:, et * SQ + sb * 512:
                                      et * SQ + (sb + 1) * 512],
                                start=(et == 0), stop=(et == ET - 1))
                        nc.scalar.activation(
                            pt[:, tt4, :], stp[:],
                            mybir.ActivationFunctionType.Exp,
                            bias=sh_sb[:], scale=SCALE)
                        # Pool-engine denominator accumulation (off the PE).
                        nc.gpsimd.tensor_add(pt_acc[:, sb, :],
                                             pt_acc[:, sb, :], pt[:, tt4, :])
                    rec = None
                    for si in range(4):
                        si_g = sb * 4 + si
                        if last and si == 2:
                            # pt_acc for this sb is final once its 4 Pool adds
                            # drain (hidden under PV si=0/1). Reduce across
                            # partitions with 4 tiny N=1 matmuls.
                            for sj in range(4):
                                nc.tensor.matmul(
                                    dn_q[:, sb * 4 + sj:sb * 4 + sj + 1],
                                    pt_acc[:, sb, sj * P:(sj + 1) * P],
                                    ones_sb[:, 0:1], start=True, stop=True)
                            rec = finp.tile([P, 4], F32, name="rec")
                            nc.vector.reciprocal(
                                rec[:], dn_q[:, sb * 4:sb * 4 + 4])
                        po = opsum.tile([P, D], F32, name="po")
                        for tt4 in range(TT4):
                            lhs = pt[:, tt4, si * P:(si + 1) * P]
                            st0 = (tt4 == 0)
                            sp1 = (tt4 == TT4 - 1)
                            nc.tensor.matmul(po[:, 0:512], lhs,
                                             v_sb[:, tt4, 0:512],
                                             start=st0, stop=sp1)
                            nc.tensor.matmul(po[:, 512:1024], lhs,
                                             v_sb[:, tt4, 512:1024],
                                             start=st0, stop=sp1)
                        if c == 0:
                            nc.vector.tensor_copy(out_acc[:, si_g, :], po[:])
                        else:
                            nc.vector.tensor_add(out_acc[:, si_g, :],
                                                 out_acc[:, si_g, :], po[:])
                    if last:
                        # normalize and stream out; sb=0's tail hides under
                        # sb=1's matmuls. sb=0 scales ride the DVE so the ACT
                        # queue stays clear for sb=1's exps; all stores go to
                        # the otherwise-idle sync ring.
                        for si in range(4):
                            si_g = sb * 4 + si
                            ost = finp.tile([P, D], BF16, name="ost")
                            if sb == 1 and si % 2 == 0:
                                nc.scalar.activation(
                                    ost[:], out_acc[:, si_g, :D],
                                    mybir.ActivationFunctionType.Copy,
                                    scale=rec[:, si:si + 1])
                            else:
                                nc.vector.tensor_scalar_mul(
                                    ost[:], out_acc[:, si_g, :D],
                                    rec[:, si:si + 1])
                            nc.sync.dma_start(
                                out=out[si_g * P:(si_g + 1) * P, :],
                                in_=ost[:])

    nc.compile()
    return nc


def _prep(hidden_states, cached_key, cached_value, Wq, bq, Wk, bk, Wv, bv):
    """Host-side resharding into SBUF-image layouts (pure reshapes/copies)."""
    def ptile_cols(a):  # [R, C] with R = n*128 -> [128, n*C] (partition-major)
        n = a.shape[0] // P
        return np.ascontiguousarray(
            a.reshape(n, P, a.shape[1]).transpose(1, 0, 2)).reshape(P, -1)

    w_h = {}
    for nm, W in (("wq", Wq), ("wk", Wk), ("wv", Wv)):
        w_h[nm] = ptile_cols(np.ascontiguousarray(W.T)).astype(NPBF16)
    bq_h = np.ascontiguousarray(bq.reshape(ET, P).T)             # [128, 8]
    bk_h = np.ascontiguousarray(bk.reshape(ET, P).T)
    bv_h = np.ascontiguousarray(np.broadcast_to(bv, (P, D)))     # [128, 1024]

    in_maps = []
    for b in range(B):
        ht_full = ptile_cols(np.ascontiguousarray(hidden_states[b].T))  # [128, 8*2048]
        # chunk-major: [chunk, P, ET, CHUNK] / [chunk, P, TT4, D]
        kcT_h = ptile_cols(np.ascontiguousarray(cached_key[b].T)) \
            .astype(NPBF16).reshape(P, ET, N_CACHED_CHUNKS, CHUNK)
        kcT_h = np.ascontiguousarray(kcT_h.transpose(2, 0, 1, 3))
        vc_h = np.ascontiguousarray(
            cached_value[b].reshape(N_CACHED_CHUNKS, TT4, P, D)
            .transpose(0, 2, 1, 3)).astype(NPBF16)
        for h in range(2):
            ht_v = ht_full.reshape(P, DT, S_NEW)
            ht_c = np.ascontiguousarray(
                ht_v[:, :, h * SQ:(h + 1) * SQ]).reshape(P, DT * SQ) \
                .astype(NPBF16)
            in_maps.append({
                "ht": ht_c, "kcT": kcT_h, "vc": vc_h,
                "wq": w_h["wq"], "wk": w_h["wk"], "wv": w_h["wv"],
                "bq": bq_h, "bk": bk_h, "bv": bv_h,
            })
    return in_maps


def kernel(hidden_states, cached_key, cached_value, Wq, bq, Wk, bk, Wv, bv,
           _trace=False):
    if "nc" not in _cache:
        _cache["nc"] = _build()
    nc = _cache["nc"]
    in_maps = _prep(
        np.asarray(hidden_states, dtype=np.float32),
        np.asarray(cached_key, dtype=np.float32),
        np.asarray(cached_value, dtype=np.float32),
        np.asarray(Wq, dtype=np.float32), np.asarray(bq, dtype=np.float32),
        np.asarray(Wk, dtype=np.float32), np.asarray(bk, dtype=np.float32),
        np.asarray(Wv, dtype=np.float32), np.asarray(bv, dtype=np.float32))
    res = run_bass_kernel_spmd(nc, in_maps, list(range(N_CORES)), trace=_trace)
    _cache["last_result"] = res
    out = np.empty((B, S_NEW, D), np.float32)
    for b in range(B):
        for h in range(2):
            out[b, h * SQ:(h + 1) * SQ, :] = \
                res.results[2 * b + h]["out"].astype(np.float32)
    return out
